# revision 2
# baseline (speedup 1.0000x reference)
"""Trainium2 Bass kernel v2 for the Engram module (hashed n-gram memory).

Contract: kernel(**inputs) takes FULL unsharded numpy inputs and returns the
FULL output (4, 2048, 2048) f32.

Sharding (hardcoded): data parallel over tokens — 8 cores x 1024 tokens
(core c -> batch c//2, seq half c%2); the 12 embedding tables replicated in
fp8 (x16 scale) in each core's DRAM; no collectives.

Device pipeline per core (window of 1026 token-cols = 2 left-context + 1024):
  phase A (9 token tiles: 8x128 + 1x2):
    - ONE multi-index indirect DMA per tile gathers 12 fp8 rows/token
    - q = hs @ W_q on PE in fp8 DoubleRow mode (2 K-planes per instr)
    - fused DVE tensor_tensor_reduce -> dot(q, mem); ACT sigmoid -> alpha
    - amb = alpha*mem (ACT, bf16); PE bf16 transposes -> amt (fp8)
  phase B (16 hid tiles):
    - causal depthwise conv FOLDED INTO the value matmul: 3 host-precomputed
      (W_v * conv_w[:,k]) matrices, 3 shifted rhs reads of amt, all
      accumulated in PSUM via fp8 DoubleRow matmuls
    - ACT scale -> bf16 out tile; batched DMA out
  Residual hs + conv bias are added on the HOST during unshard (f32).

Host computes the n-gram hash indices (integer ops) while sharding.
"""

import os
import numpy as np
import ml_dtypes

# ---------------- problem constants (hardcoded per the contract) -------------
B, S, HID = 4, 2048, 2048
TABLE, EMB = 200000, 64
ORDERS, HEADS = 3, 4
NSLOT = ORDERS * HEADS            # 12
MEMD = NSLOT * EMB                # 768
KCONV = 3
NCORES = 8
TOK = 1024                        # output tokens per core
CTX = 2                           # left context (conv taps)
NTILE = 9                         # 9 uniform 128-token tiles
WIN = 128 * NTILE                 # 1152 cols; col c <-> token t0-2+c (pad tail)
VWIN = CTX + TOK                  # 1026 valid cols
ZROW = NSLOT * TABLE              # all-zeros pad row
TABROWS = ZROW + 4
NKP = HID // 256                  # 8 K-pair planes for Q
MKP = MEMD // 256                 # 3 K-pair planes for V
NHID = HID // 128                 # 16 hid tiles

SCALE_TAB = 16.0
SCALE_WQ = 32.0
SCALE_WVK = 128.0
SIG_SCALE = 1.0 / (float(np.sqrt(np.float64(MEMD))) * SCALE_TAB * SCALE_WQ)
OUT_SCALE = 1.0 / (SCALE_TAB * SCALE_WVK)

HEAD_MULTS = np.array([2654435761, 2246822519, 3266489917, 668265263],
                      dtype=np.uint32)
POLY = np.uint32(1000003)

_BF16 = ml_dtypes.bfloat16
_FP8 = ml_dtypes.float8_e4m3


def _global_rows(input_ids: np.ndarray) -> np.ndarray:
    """(B, S) int -> (B, S, 12) int32 global row ids into the stacked table."""
    Bb, Ss = input_ids.shape
    u = input_ids.astype(np.uint32)
    per_order = []
    for n in range(2, 2 + ORDERS):
        pad = np.zeros((Bb, Ss + n - 1), np.uint32)
        pad[:, n - 1:] = u
        acc = np.zeros((Bb, Ss), np.uint32)
        for j in range(n):
            acc = acc * POLY + pad[:, j:j + Ss]
        idx = (acc[..., None] * HEAD_MULTS[None, None, :]) % np.uint32(TABLE)
        per_order.append(idx.astype(np.int32))
    gidx = np.stack(per_order, axis=2).reshape(Bb, Ss, NSLOT)
    gidx = gidx + (np.arange(NSLOT, dtype=np.int32) * TABLE)[None, None, :]
    return gidx


# ---------------- device program ---------------------------------------------
_NC_CACHE: dict = {}


def _build_nc():
    _key = "nc" + os.environ.get("KPHASE", "AB")
    if _key in _NC_CACHE:
        return _NC_CACHE[_key]

    from contextlib import ExitStack

    import concourse.bass as bass
    import concourse.mybir as mybir
    import concourse.tile as tile
    from concourse import bacc, library_config
    from concourse.masks import make_identity

    f32 = mybir.dt.float32
    bf16 = mybir.dt.bfloat16
    fp8 = mybir.dt.float8e4
    i32 = mybir.dt.int32
    MULT = mybir.AluOpType.mult
    ADD = mybir.AluOpType.add
    AF = mybir.ActivationFunctionType
    AXF = mybir.AxisListType
    DR = mybir.MatmulPerfMode.DoubleRow

    nc = bacc.Bacc("TRN2", target_bir_lowering=False, debug=False,
                   enable_asserts=False, num_devices=NCORES)

    tab = nc.dram_tensor("tab8", [TABROWS, EMB], fp8,
                         kind="ExternalInput").ap()
    hst = nc.dram_tensor("hst8", [NKP * 128, 2 * WIN], fp8,
                         kind="ExternalInput").ap()
    wq = nc.dram_tensor("wq8", [NKP * 128, 2 * MEMD], fp8,
                        kind="ExternalInput").ap()
    wvk = nc.dram_tensor("wvk8", [MKP * 128, KCONV * 2 * HID], fp8,
                         kind="ExternalInput").ap()
    idxs = nc.dram_tensor("idxs", [128, NTILE * NSLOT], i32,
                          kind="ExternalInput").ap()
    outT = nc.dram_tensor("outT", [HID, TOK], bf16, kind="ExternalOutput").ap()

    with tile.TileContext(nc) as tc, ExitStack() as ctx:
        pool = lambda name, bufs, space="SBUF": ctx.enter_context(
            tc.tile_pool(name=name, bufs=bufs, space=space))

        p_const = pool("const", 1)
        p_w = pool("w", 1)
        p_amt = pool("amt", 1)
        p_mem = pool("mem", 3)
        p_amb = pool("amb", 2)
        p_scr = pool("scr", 2)
        p_sc = pool("sc", 3)
        p_out = pool("out", 2)
        p_qps = pool("qps", 2, space="PSUM")
        p_vps = pool("vps", 2, space="PSUM")

        identb = p_const.tile([128, 128], bf16)
        make_identity(nc, identb[:])
        # resident weights (one dma_start each; wvk split by K-pair) ---------
        idx_sb = p_w.tile([128, NTILE * NSLOT], i32, name="idx_sb")
        nc.sync.dma_start(idx_sb[:], idxs[:, :])
        hst_sb = p_w.tile([128, NKP, 2, WIN], fp8, name="hst_sb")
        nc.sync.dma_start(
            hst_sb[:],
            hst.rearrange("(kp p) (pl c) -> p kp pl c", p=128, pl=2))
        wq_sb = p_w.tile([128, NKP, 2, MEMD], fp8, name="wq_sb")
        nc.sync.dma_start(
            wq_sb[:],
            wq.rearrange("(kp p) (pl n) -> p kp pl n", p=128, pl=2))
        # wvk is loaded AFTER phase A is emitted (same SP queue) so the
        # hst/wq loads that gate the first Q matmul get the DMA bandwidth.
        wvk_sb = [p_w.tile([128, KCONV, 2, HID], fp8, name=f"wvk_sb{kp}")
                  for kp in range(MKP)]

        amt_sb = p_amt.tile([128, MKP, 2, WIN], fp8, name="amt_sb")

        _phases = os.environ.get("KPHASE", "AB")
        if "A" not in _phases:
            nc.vector.memset(amt_sb[:], 0.125)

        # phase A: gather + gate + transposed alpha*mem ----------------------
        _pending = None
        _na = NTILE if ("A" in _phases or _phases in ("G", "Q")) else 0
        for i in range(_na):
            c0 = 128 * i
            w = min(128, WIN - c0)
            mem8 = p_mem.tile([128, MEMD], fp8, tag="mem", name=f"mem{i}")
            if _phases == "Q":
                nc.vector.memset(mem8[:], 0.125)
            else:
                for j in range(NSLOT):
                    nc.gpsimd.indirect_dma_start(
                        out=mem8[:, EMB * j:EMB * (j + 1)],
                        out_offset=None,
                        in_=tab[:, :],
                        in_offset=bass.IndirectOffsetOnAxis(
                            ap=idx_sb[:, NSLOT * i + j:NSLOT * i + j + 1],
                            axis=0))
            memf = p_scr.tile([128, MEMD], bf16, tag="memb", name=f"memb{i}")
            nc.scalar.activation(memf[:, :], mem8[:, :], AF.Identity)
            if _phases == "G":
                nc.scalar.activation(amt_sb[:, 0, 0, c0:c0 + w],
                                     memf[:, 0:w], AF.Identity)
                continue
            qA = p_qps.tile([128, 512], f32, space="PSUM", tag="qA", name=f"qA{i}")
            qB = p_qps.tile([128, 256], f32, space="PSUM", tag="qB", name=f"qB{i}")
            for n0 in range(0, MEMD, 256):
                qo = qA[0:w, n0:n0 + 256] if n0 < 512 else qB[0:w, 0:256]
                for kp in range(NKP):
                    nc.tensor.matmul(qo, lhsT=hst_sb[:, kp, :, c0:c0 + w],
                                     rhs=wq_sb[:, kp, :, n0:n0 + 256],
                                     start=(kp == 0), stop=(kp == NKP - 1),
                                     perf_mode=DR)
            prod = p_scr.tile([128, MEMD], f32, tag="scr", name=f"prod{i}")
            nc.vector.tensor_mul(prod[0:w, 0:512], qA[0:w, :], memf[0:w, 0:512])
            nc.vector.tensor_mul(prod[0:w, 512:768], qB[0:w, :],
                                 memf[0:w, 512:768])
            dot = p_sc.tile([128, 1], f32, tag="dot", name=f"dot{i}")
            nc.vector.tensor_reduce(dot[0:w, :], prod[0:w, :], AXF.X, ADD)
            alpha = p_sc.tile([128, 1], f32, tag="alpha", name=f"alpha{i}")
            nc.scalar.activation(alpha[0:w, :], dot[0:w, :], AF.Sigmoid,
                                 scale=SIG_SCALE)
            amb = p_amb.tile([128, MEMD], bf16, tag="amb", name=f"amb{i}")
            nc.scalar.activation(amb[0:w, :], memf[0:w, :], AF.Identity,
                                 scale=alpha[0:w, :])

            def _emit_transposes(amb, w, c0, i):
                # all 6 m-tiles transposed into ONE psum bank, one ACT copy
                tp = p_vps.tile([128, MEMD // 128, w], bf16, space="PSUM",
                                tag="v", name=f"tp{i}")
                for mt in range(MEMD // 128):
                    nc.tensor.transpose(
                        tp[:, mt, :], amb[0:w, 128 * mt:128 * (mt + 1)],
                        identb[0:w, 0:w])
                # ACT copy (DVE fp8 output is broken on HW); dest is the
                # [kp, pl] plane layout, source is m-tile-major == same order
                nc.scalar.activation(
                    amt_sb[:, :, :, c0:c0 + w].rearrange(
                        "p kp pl c -> p (kp pl) c"), tp[:, :, :],
                    AF.Identity)

            # software pipeline: transposes of tile i-1 go AFTER Q of tile i
            # in the PE queue, hiding the DVE/ACT gate latency.
            if _pending is not None:
                _emit_transposes(*_pending)
            _pending = (amb, w, c0, i)
        if _pending is not None:
            _emit_transposes(*_pending)

        for kp in range(MKP):
            nc.sync.dma_start(
                wvk_sb[kp][:],
                wvk[128 * kp:128 * (kp + 1), :].rearrange(
                    "p (k pl h) -> p k pl h", k=KCONV, pl=2))

        # phase B: value matmul with folded causal conv ----------------------
        if "B" not in _phases:
            zt = p_out.tile([128, 4, TOK], bf16, tag="out", name="zt")
            nc.vector.memset(zt[:], 0)
            for q in range(4):
                nc.sync.dma_start(
                    outT[512 * q:512 * (q + 1), :].rearrange(
                        "(sub p) c -> p sub c", p=128), zt[:, :, :])
        for mt in range(NHID if "B" in _phases else 0):
            h0 = 128 * mt
            ob = p_out.tile([128, 4, TOK], bf16, tag="out", name=f"ob{mt // 4}_{mt % 4}") \
                if mt % 4 == 0 else ob  # noqa: F821
            for half in range(2):
                pv = p_vps.tile([128, 512], f32, space="PSUM", tag="v",
                                name=f"pv{mt}_{half}")
                for ch in range(2):
                    j0 = 512 * half + 256 * ch
                    n = 0
                    for k in range(KCONV):
                        sh = k  # tap k reads g[t-2+k] = window col j+k
                        for kp in range(MKP):
                            nc.tensor.matmul(
                                pv[:, 256 * ch:256 * (ch + 1)],
                                lhsT=wvk_sb[kp][:, k, :, h0:h0 + 128],
                                rhs=amt_sb[:, kp, :, j0 + sh:j0 + sh + 256],
                                start=(n == 0), stop=(n == KCONV * MKP - 1),
                                perf_mode=DR)
                            n += 1
                nc.scalar.activation(
                    ob[:, mt % 4, 512 * half:512 * (half + 1)], pv[:, :],
                    AF.Identity, scale=OUT_SCALE)
            if mt % 4 == 3:
                nc.sync.dma_start(
                    outT[128 * (mt - 3):128 * (mt + 1), :].rearrange(
                        "(sub p) c -> p sub c", p=128),
                    ob[:, :, :])

    nc.compile()
    _NC_CACHE[_key] = nc
    return nc


# ---------------- host-side sharding -----------------------------------------
def _make_in_maps(inputs: dict):
    hs = np.asarray(inputs["hidden_states"], dtype=np.float32)
    ids = np.asarray(inputs["input_ids"])
    tabs = np.asarray(inputs["emb_tables"], dtype=np.float32)
    W_q = np.asarray(inputs["W_q"], dtype=np.float32)
    W_v = np.asarray(inputs["W_v"], dtype=np.float32)
    conv_w = np.asarray(inputs["conv_w"], dtype=np.float32).reshape(HID, KCONV)
    conv_b = np.asarray(inputs["conv_b"], dtype=np.float32)

    tab8 = np.zeros((TABROWS, EMB), dtype=_FP8)
    tab8[:ZROW] = (tabs.reshape(ZROW, EMB) * SCALE_TAB).astype(_FP8)
    gidx = _global_rows(ids)                              # (B, S, 12) int32

    # wq8[kp*128+p, pl*768+n] = 32*W_q[256kp+128pl+p, n]
    wq8 = np.ascontiguousarray(
        (W_q.reshape(NKP, 2, 128, MEMD).transpose(0, 2, 1, 3) * SCALE_WQ)
        .astype(_FP8).reshape(NKP * 128, 2 * MEMD))
    # wvk8[kp*128+p, (k*2+pl)*2048+h] = 128*W_v[256kp+128pl+p, h]*conv_w[h,k]
    wvk = (W_v[None, :, :] * conv_w.T[:, None, :] * SCALE_WVK)  # (3, 768, 2048)
    wvk8 = np.ascontiguousarray(
        wvk.reshape(KCONV, MKP, 2, 128, HID).transpose(1, 3, 0, 2, 4)
        .astype(_FP8).reshape(MKP * 128, KCONV * 2 * HID))

    in_maps = []
    for c in range(NCORES):
        b, h = divmod(c, 2)
        t0 = h * TOK
        lo = t0 - CTX
        v0 = max(0, lo)                                   # first valid token
        nv = t0 + TOK - v0                                # valid token count
        win_idx = np.full((WIN, NSLOT), ZROW, dtype=np.int32)
        win_idx[v0 - lo:v0 - lo + nv] = gidx[b, v0:t0 + TOK]
        hsw = np.zeros((WIN, HID), dtype=np.float32)
        hsw[v0 - lo:v0 - lo + nv] = hs[b, v0:t0 + TOK]
        # hst8[kp*128+p, pl*WIN+c] = hs[g(c), 256kp+128pl+p]
        hst8 = np.ascontiguousarray(
            hsw.reshape(WIN, NKP, 2, 128).transpose(1, 3, 2, 0)
            .astype(_FP8).reshape(NKP * 128, 2 * WIN))
        in_maps.append({
            "tab8": tab8,
            "hst8": hst8,
            "wq8": wq8,
            "wvk8": wvk8,
            "idxs": np.ascontiguousarray(
                win_idx.reshape(NTILE, 128, NSLOT).transpose(1, 0, 2)
                .reshape(128, NTILE * NSLOT)),
        })
    return in_maps


def _postprocess_core(outT_np: np.ndarray, inputs: dict, c: int) -> np.ndarray:
    """Device outT (HID, TOK) bf16 fused -> full (TOK, HID) f32 output slice."""
    hs = np.asarray(inputs["hidden_states"], dtype=np.float32)
    cb = np.asarray(inputs["conv_b"], dtype=np.float32)
    b, h = divmod(c, 2)
    t0 = h * TOK
    return hs[b, t0:t0 + TOK, :] + outT_np.astype(np.float32).T + cb


def _run(inputs: dict, trace: bool = False, **kw):
    from concourse import bass_utils

    nc = _build_nc()
    in_maps = _make_in_maps(inputs)
    res = bass_utils.run_bass_kernel_spmd(
        nc, in_maps, core_ids=list(range(NCORES)), trace=trace, **kw)
    out = np.empty((B, S, HID), dtype=np.float32)
    for c in range(NCORES):
        b, h = divmod(c, 2)
        out[b, h * TOK:(h + 1) * TOK, :] = _postprocess_core(
            res.results[c]["outT"], inputs, c)
    return out, res


def kernel(**inputs) -> np.ndarray:
    out, _ = _run(inputs, trace=False)
    return out


# revision 3
# speedup vs baseline: 1.0011x; 1.0011x over previous
"""Trainium2 Bass kernel v2 for the Engram module (hashed n-gram memory).

Contract: kernel(**inputs) takes FULL unsharded numpy inputs and returns the
FULL output (4, 2048, 2048) f32.

Sharding (hardcoded): data parallel over tokens — 8 cores x 1024 tokens
(core c -> batch c//2, seq half c%2); the 12 embedding tables replicated in
fp8 (x16 scale) in each core's DRAM; no collectives.

Device pipeline per core (window of 1026 token-cols = 2 left-context + 1024):
  phase A (9 token tiles: 8x128 + 1x2):
    - ONE multi-index indirect DMA per tile gathers 12 fp8 rows/token
    - q = hs @ W_q on PE in fp8 DoubleRow mode (2 K-planes per instr)
    - fused DVE tensor_tensor_reduce -> dot(q, mem); ACT sigmoid -> alpha
    - amb = alpha*mem (ACT, bf16); PE bf16 transposes -> amt (fp8)
  phase B (16 hid tiles):
    - causal depthwise conv FOLDED INTO the value matmul: 3 host-precomputed
      (W_v * conv_w[:,k]) matrices, 3 shifted rhs reads of amt, all
      accumulated in PSUM via fp8 DoubleRow matmuls
    - ACT scale -> bf16 out tile; batched DMA out
  Residual hs + conv bias are added on the HOST during unshard (f32).

Host computes the n-gram hash indices (integer ops) while sharding.
"""

import os
import numpy as np
import ml_dtypes

# ---------------- problem constants (hardcoded per the contract) -------------
B, S, HID = 4, 2048, 2048
TABLE, EMB = 200000, 64
ORDERS, HEADS = 3, 4
NSLOT = ORDERS * HEADS            # 12
MEMD = NSLOT * EMB                # 768
KCONV = 3
NCORES = 8
TOK = 1024                        # output tokens per core
CTX = 2                           # left context (conv taps)
NTILE = 9                         # 9 uniform 128-token tiles
WIN = 128 * NTILE                 # 1152 cols; col c <-> token t0-2+c (pad tail)
VWIN = CTX + TOK                  # 1026 valid cols
ZROW = NSLOT * TABLE              # all-zeros pad row
TABROWS = ZROW + 4
NKP = HID // 256                  # 8 K-pair planes for Q
MKP = MEMD // 256                 # 3 K-pair planes for V
NHID = HID // 128                 # 16 hid tiles

SCALE_TAB = 16.0
SCALE_WQ = 32.0
SCALE_WVK = 128.0
SIG_SCALE = 1.0 / (float(np.sqrt(np.float64(MEMD))) * SCALE_TAB * SCALE_WQ)
OUT_SCALE = 1.0 / (SCALE_TAB * SCALE_WVK)

HEAD_MULTS = np.array([2654435761, 2246822519, 3266489917, 668265263],
                      dtype=np.uint32)
POLY = np.uint32(1000003)

_BF16 = ml_dtypes.bfloat16
_FP8 = ml_dtypes.float8_e4m3


def _global_rows(input_ids: np.ndarray) -> np.ndarray:
    """(B, S) int -> (B, S, 12) int32 global row ids into the stacked table."""
    Bb, Ss = input_ids.shape
    u = input_ids.astype(np.uint32)
    per_order = []
    for n in range(2, 2 + ORDERS):
        pad = np.zeros((Bb, Ss + n - 1), np.uint32)
        pad[:, n - 1:] = u
        acc = np.zeros((Bb, Ss), np.uint32)
        for j in range(n):
            acc = acc * POLY + pad[:, j:j + Ss]
        idx = (acc[..., None] * HEAD_MULTS[None, None, :]) % np.uint32(TABLE)
        per_order.append(idx.astype(np.int32))
    gidx = np.stack(per_order, axis=2).reshape(Bb, Ss, NSLOT)
    gidx = gidx + (np.arange(NSLOT, dtype=np.int32) * TABLE)[None, None, :]
    return gidx


# ---------------- device program ---------------------------------------------
_NC_CACHE: dict = {}


def _build_nc():
    _key = "nc" + os.environ.get("KPHASE", "AB")
    if _key in _NC_CACHE:
        return _NC_CACHE[_key]

    from contextlib import ExitStack

    import concourse.bass as bass
    import concourse.mybir as mybir
    import concourse.tile as tile
    from concourse import bacc, library_config
    from concourse.masks import make_identity

    f32 = mybir.dt.float32
    bf16 = mybir.dt.bfloat16
    fp8 = mybir.dt.float8e4
    i32 = mybir.dt.int32
    MULT = mybir.AluOpType.mult
    ADD = mybir.AluOpType.add
    AF = mybir.ActivationFunctionType
    AXF = mybir.AxisListType
    DR = mybir.MatmulPerfMode.DoubleRow

    nc = bacc.Bacc("TRN2", target_bir_lowering=False, debug=False,
                   enable_asserts=False, num_devices=NCORES)

    tab = nc.dram_tensor("tab8", [TABROWS, EMB], fp8,
                         kind="ExternalInput").ap()
    hst = nc.dram_tensor("hst8", [NKP * 128, 2 * WIN], fp8,
                         kind="ExternalInput").ap()
    wq = nc.dram_tensor("wq8", [NKP * 128, 2 * MEMD], fp8,
                        kind="ExternalInput").ap()
    wvk = nc.dram_tensor("wvk8", [MKP * 128, KCONV * 2 * HID], fp8,
                         kind="ExternalInput").ap()
    idxs = nc.dram_tensor("idxs", [128, NTILE * NSLOT], i32,
                          kind="ExternalInput").ap()
    outT = nc.dram_tensor("outT", [HID, TOK], bf16, kind="ExternalOutput").ap()

    with tile.TileContext(nc) as tc, ExitStack() as ctx:
        pool = lambda name, bufs, space="SBUF": ctx.enter_context(
            tc.tile_pool(name=name, bufs=bufs, space=space))

        p_const = pool("const", 1)
        p_w = pool("w", 1)
        p_amt = pool("amt", 1)
        p_mem = pool("mem", 3)
        p_amb = pool("amb", 2)
        p_scr = pool("scr", 2)
        p_sc = pool("sc", 3)
        p_out = pool("out", 2)
        p_qps = pool("qps", 2, space="PSUM")
        p_vps = pool("vps", 2, space="PSUM")

        identb = p_const.tile([128, 128], bf16)
        make_identity(nc, identb[:])
        # resident weights (one dma_start each; wvk split by K-pair) ---------
        idx_sb = p_w.tile([128, NTILE * NSLOT], i32, name="idx_sb")
        nc.sync.dma_start(idx_sb[:], idxs[:, :])
        hst_sb = p_w.tile([128, NKP, 2, WIN], fp8, name="hst_sb")
        nc.sync.dma_start(
            hst_sb[:],
            hst.rearrange("(kp p) (pl c) -> p kp pl c", p=128, pl=2))
        wq_sb = p_w.tile([128, NKP, 2, MEMD], fp8, name="wq_sb")
        nc.sync.dma_start(
            wq_sb[:],
            wq.rearrange("(kp p) (pl n) -> p kp pl n", p=128, pl=2))
        # wvk is loaded AFTER phase A is emitted (same SP queue) so the
        # hst/wq loads that gate the first Q matmul get the DMA bandwidth.
        wvk_sb = []
        for kp in range(MKP):
            t = p_w.tile([128, KCONV, 2, HID], fp8, name=f"wvk_sb{kp}")
            nc.sync.dma_start(
                t[:],
                wvk[128 * kp:128 * (kp + 1), :].rearrange(
                    "p (k pl h) -> p k pl h", k=KCONV, pl=2))
            wvk_sb.append(t)

        # window split for A/B overlap: L = cols [0,640) (tiles 0-4),
        # R = cols [512,1152) (tiles 4-8); tile 4 writes both.
        amtL = p_amt.tile([128, MKP, 2, 640], fp8, name="amtL")
        amtR = p_amt.tile([128, MKP, 2, 640], fp8, name="amtR")

        _phases = os.environ.get("KPHASE", "AB")
        if "A" not in _phases:
            nc.vector.memset(amtL[:], 0.125)
            nc.vector.memset(amtR[:], 0.125)

        def _emit_bpass(p):
            src_t = amtL if p == 0 else amtR
            for mt in range(NHID):
                h0 = 128 * mt
                ob = p_out.tile([128, 4, 512], bf16, tag=f"out{p}",
                                name=f"ob{p}_{mt // 4}_{mt % 4}") \
                    if mt % 4 == 0 else ob  # noqa: F821
                pv = p_vps.tile([128, 512], f32, space="PSUM", tag="v",
                                name=f"pv{p}_{mt}")
                for ch in range(2):
                    j0 = 256 * ch
                    n = 0
                    for k in range(KCONV):
                        for kp in range(MKP):
                            nc.tensor.matmul(
                                pv[:, 256 * ch:256 * (ch + 1)],
                                lhsT=wvk_sb[kp][:, k, :, h0:h0 + 128],
                                rhs=src_t[:, kp, :, j0 + k:j0 + k + 256],
                                start=(n == 0), stop=(n == KCONV * MKP - 1),
                                perf_mode=DR)
                            n += 1
                nc.scalar.activation(ob[:, mt % 4, :], pv[:, :],
                                     AF.Identity, scale=OUT_SCALE)
                if mt % 4 == 3:
                    nc.sync.dma_start(
                        outT[512 * (mt // 4):512 * (mt // 4 + 1),
                             512 * p:512 * (p + 1)].rearrange(
                            "(sub p2) c -> p2 sub c", p2=128),
                        ob[:, :, :])

        # phase A: gather + gate + transposed alpha*mem ----------------------
        _pending = None
        _na = NTILE if ("A" in _phases or _phases in ("G", "Q")) else 0
        for i in range(_na):
            c0 = 128 * i
            w = min(128, WIN - c0)
            mem8 = p_mem.tile([128, MEMD], fp8, tag="mem", name=f"mem{i}")
            if _phases == "Q":
                nc.vector.memset(mem8[:], 0.125)
            else:
                for j in range(NSLOT):
                    nc.gpsimd.indirect_dma_start(
                        out=mem8[:, EMB * j:EMB * (j + 1)],
                        out_offset=None,
                        in_=tab[:, :],
                        in_offset=bass.IndirectOffsetOnAxis(
                            ap=idx_sb[:, NSLOT * i + j:NSLOT * i + j + 1],
                            axis=0))
            memf = p_scr.tile([128, MEMD], bf16, tag="memb", name=f"memb{i}")
            nc.scalar.activation(memf[:, :], mem8[:, :], AF.Identity)
            if _phases == "G":
                nc.scalar.activation(amt_sb[:, 0, 0, c0:c0 + w],
                                     memf[:, 0:w], AF.Identity)
                continue
            qA = p_qps.tile([128, 512], f32, space="PSUM", tag="qA", name=f"qA{i}")
            qB = p_qps.tile([128, 256], f32, space="PSUM", tag="qB", name=f"qB{i}")
            for n0 in range(0, MEMD, 256):
                qo = qA[0:w, n0:n0 + 256] if n0 < 512 else qB[0:w, 0:256]
                for kp in range(NKP):
                    nc.tensor.matmul(qo, lhsT=hst_sb[:, kp, :, c0:c0 + w],
                                     rhs=wq_sb[:, kp, :, n0:n0 + 256],
                                     start=(kp == 0), stop=(kp == NKP - 1),
                                     perf_mode=DR)
            prod = p_scr.tile([128, MEMD], f32, tag="scr", name=f"prod{i}")
            nc.vector.tensor_mul(prod[0:w, 0:512], qA[0:w, :], memf[0:w, 0:512])
            nc.vector.tensor_mul(prod[0:w, 512:768], qB[0:w, :],
                                 memf[0:w, 512:768])
            dot = p_sc.tile([128, 1], f32, tag="dot", name=f"dot{i}")
            nc.vector.tensor_reduce(dot[0:w, :], prod[0:w, :], AXF.X, ADD)
            alpha = p_sc.tile([128, 1], f32, tag="alpha", name=f"alpha{i}")
            nc.scalar.activation(alpha[0:w, :], dot[0:w, :], AF.Sigmoid,
                                 scale=SIG_SCALE)
            amb = p_amb.tile([128, MEMD], bf16, tag="amb", name=f"amb{i}")
            nc.scalar.activation(amb[0:w, :], memf[0:w, :], AF.Identity,
                                 scale=alpha[0:w, :])

            def _emit_transposes(amb, w, c0, i):
                # all 6 m-tiles transposed into ONE psum bank, one ACT copy
                tp = p_vps.tile([128, MEMD // 128, w], bf16, space="PSUM",
                                tag="v", name=f"tp{i}")
                for mt in range(MEMD // 128):
                    nc.tensor.transpose(
                        tp[:, mt, :], amb[0:w, 128 * mt:128 * (mt + 1)],
                        identb[0:w, 0:w])
                # ACT copy (DVE fp8 output is broken on HW); dest is the
                # [kp, pl] plane layout, source is m-tile-major == same order
                targets = []
                if i <= 4:
                    targets.append((amtL, c0))
                if i >= 4:
                    targets.append((amtR, c0 - 512))
                for dst, cc in targets:
                    nc.scalar.activation(
                        dst[:, :, :, cc:cc + w].rearrange(
                            "p kp pl c -> p (kp pl) c"), tp[:, :, :],
                        AF.Identity)

            # software pipeline: transposes of tile i-1 go AFTER Q of tile i
            # in the PE queue, hiding the DVE/ACT gate latency.
            if _pending is not None:
                _emit_transposes(*_pending)
            _pending = (amb, w, c0, i)
            if i == 5 and "B" in _phases:
                _emit_bpass(0)   # left half overlaps tiles 5-8 gathers
        if _pending is not None:
            _emit_transposes(*_pending)
        if "B" in _phases:
            _emit_bpass(1)

    nc.compile()
    _NC_CACHE[_key] = nc
    return nc


# ---------------- host-side sharding -----------------------------------------
def _make_in_maps(inputs: dict):
    hs = np.asarray(inputs["hidden_states"], dtype=np.float32)
    ids = np.asarray(inputs["input_ids"])
    tabs = np.asarray(inputs["emb_tables"], dtype=np.float32)
    W_q = np.asarray(inputs["W_q"], dtype=np.float32)
    W_v = np.asarray(inputs["W_v"], dtype=np.float32)
    conv_w = np.asarray(inputs["conv_w"], dtype=np.float32).reshape(HID, KCONV)
    conv_b = np.asarray(inputs["conv_b"], dtype=np.float32)

    tab8 = np.zeros((TABROWS, EMB), dtype=_FP8)
    tab8[:ZROW] = (tabs.reshape(ZROW, EMB) * SCALE_TAB).astype(_FP8)
    gidx = _global_rows(ids)                              # (B, S, 12) int32

    # wq8[kp*128+p, pl*768+n] = 32*W_q[256kp+128pl+p, n]
    wq8 = np.ascontiguousarray(
        (W_q.reshape(NKP, 2, 128, MEMD).transpose(0, 2, 1, 3) * SCALE_WQ)
        .astype(_FP8).reshape(NKP * 128, 2 * MEMD))
    # wvk8[kp*128+p, (k*2+pl)*2048+h] = 128*W_v[256kp+128pl+p, h]*conv_w[h,k]
    wvk = (W_v[None, :, :] * conv_w.T[:, None, :] * SCALE_WVK)  # (3, 768, 2048)
    wvk8 = np.ascontiguousarray(
        wvk.reshape(KCONV, MKP, 2, 128, HID).transpose(1, 3, 0, 2, 4)
        .astype(_FP8).reshape(MKP * 128, KCONV * 2 * HID))

    in_maps = []
    for c in range(NCORES):
        b, h = divmod(c, 2)
        t0 = h * TOK
        lo = t0 - CTX
        v0 = max(0, lo)                                   # first valid token
        nv = t0 + TOK - v0                                # valid token count
        win_idx = np.full((WIN, NSLOT), ZROW, dtype=np.int32)
        win_idx[v0 - lo:v0 - lo + nv] = gidx[b, v0:t0 + TOK]
        hsw = np.zeros((WIN, HID), dtype=np.float32)
        hsw[v0 - lo:v0 - lo + nv] = hs[b, v0:t0 + TOK]
        # hst8[kp*128+p, pl*WIN+c] = hs[g(c), 256kp+128pl+p]
        hst8 = np.ascontiguousarray(
            hsw.reshape(WIN, NKP, 2, 128).transpose(1, 3, 2, 0)
            .astype(_FP8).reshape(NKP * 128, 2 * WIN))
        in_maps.append({
            "tab8": tab8,
            "hst8": hst8,
            "wq8": wq8,
            "wvk8": wvk8,
            "idxs": np.ascontiguousarray(
                win_idx.reshape(NTILE, 128, NSLOT).transpose(1, 0, 2)
                .reshape(128, NTILE * NSLOT)),
        })
    return in_maps


def _postprocess_core(outT_np: np.ndarray, inputs: dict, c: int) -> np.ndarray:
    """Device outT (HID, TOK) bf16 fused -> full (TOK, HID) f32 output slice."""
    hs = np.asarray(inputs["hidden_states"], dtype=np.float32)
    cb = np.asarray(inputs["conv_b"], dtype=np.float32)
    b, h = divmod(c, 2)
    t0 = h * TOK
    return hs[b, t0:t0 + TOK, :] + outT_np.astype(np.float32).T + cb


def _run(inputs: dict, trace: bool = False, **kw):
    from concourse import bass_utils

    nc = _build_nc()
    in_maps = _make_in_maps(inputs)
    res = bass_utils.run_bass_kernel_spmd(
        nc, in_maps, core_ids=list(range(NCORES)), trace=trace, **kw)
    out = np.empty((B, S, HID), dtype=np.float32)
    for c in range(NCORES):
        b, h = divmod(c, 2)
        out[b, h * TOK:(h + 1) * TOK, :] = _postprocess_core(
            res.results[c]["outT"], inputs, c)
    return out, res


def kernel(**inputs) -> np.ndarray:
    out, _ = _run(inputs, trace=False)
    return out


# revision 4
# speedup vs baseline: 1.0346x; 1.0335x over previous
"""Trainium2 Bass kernel for the Engram module (hashed n-gram memory).

Contract: kernel(**inputs) takes FULL unsharded numpy inputs and returns the
FULL output (4, 2048, 2048) f32.

Sharding (hardcoded): data parallel over tokens — 8 cores x 1024 tokens
(core c -> batch c//2, seq half c%2); the 12 embedding tables replicated in
fp8 (x16 scale) in each core's DRAM; no collectives. Host computes the
n-gram hash indices (integer ops) while sharding; host also adds the f32
residual + conv bias during unshard.

Device pipeline per core (window of 1152 cols = 2 left-context + 1024 + pad):
  phase A (9 token tiles of 128):
    - 12 single-index indirect DMAs per tile gather the fp8 embedding rows
      (HW supports only one index per partition per indirect DMA)
    - q = hs @ W_q on PE, fp8 DoubleRow mode (2 K-planes per instruction)
    - dot(q, mem) via DVE mul+reduce; ACT sigmoid -> alpha; amb = alpha*mem
    - PE bf16 transposes into one PSUM bank; single ACT copy -> amt (fp8)
  phase B (2 passes x 16 hid tiles), causal conv FOLDED into the value
  matmul: 3 host-precomputed (W_v * conv_w[:,k]) fp8 matrices, 3 shifted
  rhs reads of amt, accumulated in PSUM via fp8 DoubleRow matmuls; ACT
  scale -> bf16 out; batched DMA out. The window is split L/R so the left
  pass's PE work overlaps the right half's gathers (Pool-engine bound).
"""

import os
import numpy as np
import ml_dtypes

# ---------------- problem constants (hardcoded per the contract) -------------
B, S, HID = 4, 2048, 2048
TABLE, EMB = 200000, 64
ORDERS, HEADS = 3, 4
NSLOT = ORDERS * HEADS            # 12
MEMD = NSLOT * EMB                # 768
KCONV = 3
NCORES = 8
TOK = 1024                        # output tokens per core
CTX = 2                           # left context (conv taps)
NTILE = 9                         # 9 uniform 128-token tiles
WIN = 128 * NTILE                 # 1152 cols; col c <-> token t0-2+c (pad tail)
VWIN = CTX + TOK                  # 1026 valid cols
ZROW = NSLOT * TABLE              # all-zeros pad row
TABROWS = ZROW + 4
NKP = HID // 256                  # 8 K-pair planes for Q
MKP = MEMD // 256                 # 3 K-pair planes for V
NHID = HID // 128                 # 16 hid tiles

SCALE_TAB = 16.0
SCALE_WQ = 32.0
SCALE_WVK = 128.0
SIG_SCALE = 1.0 / (float(np.sqrt(np.float64(MEMD))) * SCALE_TAB * SCALE_WQ)
OUT_SCALE = 1.0 / (SCALE_TAB * SCALE_WVK)

HEAD_MULTS = np.array([2654435761, 2246822519, 3266489917, 668265263],
                      dtype=np.uint32)
POLY = np.uint32(1000003)

_BF16 = ml_dtypes.bfloat16
_FP8 = ml_dtypes.float8_e4m3


def _global_rows(input_ids: np.ndarray) -> np.ndarray:
    """(B, S) int -> (B, S, 12) int32 global row ids into the stacked table."""
    Bb, Ss = input_ids.shape
    u = input_ids.astype(np.uint32)
    per_order = []
    for n in range(2, 2 + ORDERS):
        pad = np.zeros((Bb, Ss + n - 1), np.uint32)
        pad[:, n - 1:] = u
        acc = np.zeros((Bb, Ss), np.uint32)
        for j in range(n):
            acc = acc * POLY + pad[:, j:j + Ss]
        idx = (acc[..., None] * HEAD_MULTS[None, None, :]) % np.uint32(TABLE)
        per_order.append(idx.astype(np.int32))
    gidx = np.stack(per_order, axis=2).reshape(Bb, Ss, NSLOT)
    gidx = gidx + (np.arange(NSLOT, dtype=np.int32) * TABLE)[None, None, :]
    return gidx


# ---------------- device program ---------------------------------------------
_NC_CACHE: dict = {}


def _build_nc():
    _key = "nc" + os.environ.get("KPHASE", "AB")
    if _key in _NC_CACHE:
        return _NC_CACHE[_key]

    from contextlib import ExitStack

    import concourse.bass as bass
    import concourse.mybir as mybir
    import concourse.tile as tile
    from concourse import bacc, library_config
    from concourse.masks import make_identity

    f32 = mybir.dt.float32
    bf16 = mybir.dt.bfloat16
    fp8 = mybir.dt.float8e4
    i32 = mybir.dt.int32
    MULT = mybir.AluOpType.mult
    ADD = mybir.AluOpType.add
    AF = mybir.ActivationFunctionType
    AXF = mybir.AxisListType
    DR = mybir.MatmulPerfMode.DoubleRow

    nc = bacc.Bacc("TRN2", target_bir_lowering=False, debug=False,
                   enable_asserts=False, num_devices=NCORES)

    tab = nc.dram_tensor("tab8", [TABROWS, EMB], fp8,
                         kind="ExternalInput").ap()
    hst = nc.dram_tensor("hst8", [NKP * 128, 2 * WIN], fp8,
                         kind="ExternalInput").ap()
    wq = nc.dram_tensor("wq8", [NKP * 128, 2 * MEMD], fp8,
                        kind="ExternalInput").ap()
    wvk = nc.dram_tensor("wvk8", [MKP * 128, KCONV * 2 * HID], fp8,
                         kind="ExternalInput").ap()
    idxs = nc.dram_tensor("idxs", [128, NTILE * NSLOT], i32,
                          kind="ExternalInput").ap()
    outT = nc.dram_tensor("outT", [HID, TOK], bf16, kind="ExternalOutput").ap()

    with tile.TileContext(nc) as tc, ExitStack() as ctx:
        pool = lambda name, bufs, space="SBUF": ctx.enter_context(
            tc.tile_pool(name=name, bufs=bufs, space=space))

        p_const = pool("const", 1)
        p_w = pool("w", 1)
        p_amt = pool("amt", 1)
        p_mem = pool("mem", 6)
        p_amb = pool("amb", 2)
        p_scr = pool("scr", 2)
        p_sc = pool("sc", 3)
        p_out = pool("out", 2)
        p_qps = pool("qps", 2, space="PSUM")
        p_vps = pool("vps", 2, space="PSUM")

        identb = p_const.tile([128, 128], bf16)
        make_identity(nc, identb[:])
        # resident weights (one dma_start each; wvk split by K-pair) ---------
        idx_sb = p_w.tile([128, NTILE * NSLOT], i32, name="idx_sb")
        nc.sync.dma_start(idx_sb[:], idxs[:, :])
        hst_sb = p_w.tile([128, NKP, 2, WIN], fp8, name="hst_sb")
        nc.sync.dma_start(
            hst_sb[:],
            hst.rearrange("(kp p) (pl c) -> p kp pl c", p=128, pl=2))
        wq_sb = p_w.tile([128, NKP, 2, MEMD], fp8, name="wq_sb")
        nc.sync.dma_start(
            wq_sb[:],
            wq.rearrange("(kp p) (pl n) -> p kp pl n", p=128, pl=2))
        # wvk is loaded AFTER phase A is emitted (same SP queue) so the
        # hst/wq loads that gate the first Q matmul get the DMA bandwidth.
        wvk_sb = []
        for kp in range(MKP):
            t = p_w.tile([128, KCONV, 2, HID], fp8, name=f"wvk_sb{kp}")
            nc.sync.dma_start(
                t[:],
                wvk[128 * kp:128 * (kp + 1), :].rearrange(
                    "p (k pl h) -> p k pl h", k=KCONV, pl=2))
            wvk_sb.append(t)

        # window split for A/B overlap: L = cols [0,640) (tiles 0-4),
        # R = cols [512,1152) (tiles 4-8); tile 4 writes both.
        amtL = p_amt.tile([128, MKP, 2, 640], fp8, name="amtL")
        amtR = p_amt.tile([128, MKP, 2, 640], fp8, name="amtR")

        _phases = os.environ.get("KPHASE", "AB")
        if "A" not in _phases:
            nc.vector.memset(amtL[:], 0.125)
            nc.vector.memset(amtR[:], 0.125)

        def _emit_bpass(p):
            src_t = amtL if p == 0 else amtR
            for mt in range(NHID):
                h0 = 128 * mt
                ob = p_out.tile([128, 4, 512], bf16, tag=f"out{p}",
                                name=f"ob{p}_{mt // 4}_{mt % 4}") \
                    if mt % 4 == 0 else ob  # noqa: F821
                pv = p_vps.tile([128, 512], f32, space="PSUM", tag="v",
                                name=f"pv{p}_{mt}")
                for ch in range(2):
                    j0 = 256 * ch
                    n = 0
                    for k in range(KCONV):
                        for kp in range(MKP):
                            nc.tensor.matmul(
                                pv[:, 256 * ch:256 * (ch + 1)],
                                lhsT=wvk_sb[kp][:, k, :, h0:h0 + 128],
                                rhs=src_t[:, kp, :, j0 + k:j0 + k + 256],
                                start=(n == 0), stop=(n == KCONV * MKP - 1),
                                perf_mode=DR)
                            n += 1
                nc.scalar.activation(ob[:, mt % 4, :], pv[:, :],
                                     AF.Identity, scale=OUT_SCALE)
                if mt % 4 == 3:
                    nc.sync.dma_start(
                        outT[512 * (mt // 4):512 * (mt // 4 + 1),
                             512 * p:512 * (p + 1)].rearrange(
                            "(sub p2) c -> p2 sub c", p2=128),
                        ob[:, :, :])

        # phase A: gather + gate + transposed alpha*mem ----------------------
        _pending = None
        _na = NTILE if ("A" in _phases or _phases in ("G", "Q")) else 0
        for i in range(_na):
            c0 = 128 * i
            w = min(128, WIN - c0)
            mem8 = p_mem.tile([128, MEMD], fp8, tag="mem", name=f"mem{i}")
            if _phases == "Q":
                nc.vector.memset(mem8[:], 0.125)
            else:
                for j in range(NSLOT):
                    nc.gpsimd.indirect_dma_start(
                        out=mem8[:, EMB * j:EMB * (j + 1)],
                        out_offset=None,
                        in_=tab[:, :],
                        in_offset=bass.IndirectOffsetOnAxis(
                            ap=idx_sb[:, NSLOT * i + j:NSLOT * i + j + 1],
                            axis=0))
            memf = p_scr.tile([128, MEMD], bf16, tag="memb", name=f"memb{i}")
            nc.scalar.activation(memf[:, :], mem8[:, :], AF.Identity)
            if _phases == "G":
                nc.scalar.activation(amt_sb[:, 0, 0, c0:c0 + w],
                                     memf[:, 0:w], AF.Identity)
                continue
            qA = p_qps.tile([128, 512], f32, space="PSUM", tag="qA", name=f"qA{i}")
            qB = p_qps.tile([128, 256], f32, space="PSUM", tag="qB", name=f"qB{i}")
            for n0 in range(0, MEMD, 256):
                qo = qA[0:w, n0:n0 + 256] if n0 < 512 else qB[0:w, 0:256]
                for kp in range(NKP):
                    nc.tensor.matmul(qo, lhsT=hst_sb[:, kp, :, c0:c0 + w],
                                     rhs=wq_sb[:, kp, :, n0:n0 + 256],
                                     start=(kp == 0), stop=(kp == NKP - 1),
                                     perf_mode=DR)
            prod = p_scr.tile([128, MEMD], f32, tag="scr", name=f"prod{i}")
            nc.vector.tensor_mul(prod[0:w, 0:512], qA[0:w, :], memf[0:w, 0:512])
            nc.vector.tensor_mul(prod[0:w, 512:768], qB[0:w, :],
                                 memf[0:w, 512:768])
            dot = p_sc.tile([128, 1], f32, tag="dot", name=f"dot{i}")
            nc.vector.tensor_reduce(dot[0:w, :], prod[0:w, :], AXF.X, ADD)
            alpha = p_sc.tile([128, 1], f32, tag="alpha", name=f"alpha{i}")
            nc.scalar.activation(alpha[0:w, :], dot[0:w, :], AF.Sigmoid,
                                 scale=SIG_SCALE)
            amb = p_amb.tile([128, MEMD], bf16, tag="amb", name=f"amb{i}")
            nc.scalar.activation(amb[0:w, :], memf[0:w, :], AF.Identity,
                                 scale=alpha[0:w, :])

            def _emit_transposes(amb, w, c0, i):
                # all 6 m-tiles transposed into ONE psum bank, one ACT copy
                tp = p_vps.tile([128, MEMD // 128, w], bf16, space="PSUM",
                                tag="v", name=f"tp{i}")
                for mt in range(MEMD // 128):
                    nc.tensor.transpose(
                        tp[:, mt, :], amb[0:w, 128 * mt:128 * (mt + 1)],
                        identb[0:w, 0:w])
                # ACT copy (DVE fp8 output is broken on HW); dest is the
                # [kp, pl] plane layout, source is m-tile-major == same order
                targets = []
                if i <= 4:
                    targets.append((amtL, c0))
                if i >= 4:
                    targets.append((amtR, c0 - 512))
                for dst, cc in targets:
                    nc.scalar.activation(
                        dst[:, :, :, cc:cc + w].rearrange(
                            "p kp pl c -> p (kp pl) c"), tp[:, :, :],
                        AF.Identity)

            # software pipeline: transposes of tile i-1 go AFTER Q of tile i
            # in the PE queue, hiding the DVE/ACT gate latency.
            if _pending is not None:
                _emit_transposes(*_pending)
            _pending = (amb, w, c0, i)
            if i == 5 and "B" in _phases:
                _emit_bpass(0)   # left half overlaps tiles 5-8 gathers
        if _pending is not None:
            _emit_transposes(*_pending)
        if "B" in _phases:
            _emit_bpass(1)

    nc.compile()
    _NC_CACHE[_key] = nc
    return nc


# ---------------- host-side sharding -----------------------------------------
def _make_in_maps(inputs: dict):
    hs = np.asarray(inputs["hidden_states"], dtype=np.float32)
    ids = np.asarray(inputs["input_ids"])
    tabs = np.asarray(inputs["emb_tables"], dtype=np.float32)
    W_q = np.asarray(inputs["W_q"], dtype=np.float32)
    W_v = np.asarray(inputs["W_v"], dtype=np.float32)
    conv_w = np.asarray(inputs["conv_w"], dtype=np.float32).reshape(HID, KCONV)
    conv_b = np.asarray(inputs["conv_b"], dtype=np.float32)

    tab8 = np.zeros((TABROWS, EMB), dtype=_FP8)
    tab8[:ZROW] = (tabs.reshape(ZROW, EMB) * SCALE_TAB).astype(_FP8)
    gidx = _global_rows(ids)                              # (B, S, 12) int32

    # wq8[kp*128+p, pl*768+n] = 32*W_q[256kp+128pl+p, n]
    wq8 = np.ascontiguousarray(
        (W_q.reshape(NKP, 2, 128, MEMD).transpose(0, 2, 1, 3) * SCALE_WQ)
        .astype(_FP8).reshape(NKP * 128, 2 * MEMD))
    # wvk8[kp*128+p, (k*2+pl)*2048+h] = 128*W_v[256kp+128pl+p, h]*conv_w[h,k]
    wvk = (W_v[None, :, :] * conv_w.T[:, None, :] * SCALE_WVK)  # (3, 768, 2048)
    wvk8 = np.ascontiguousarray(
        wvk.reshape(KCONV, MKP, 2, 128, HID).transpose(1, 3, 0, 2, 4)
        .astype(_FP8).reshape(MKP * 128, KCONV * 2 * HID))

    in_maps = []
    for c in range(NCORES):
        b, h = divmod(c, 2)
        t0 = h * TOK
        lo = t0 - CTX
        v0 = max(0, lo)                                   # first valid token
        nv = t0 + TOK - v0                                # valid token count
        win_idx = np.full((WIN, NSLOT), ZROW, dtype=np.int32)
        win_idx[v0 - lo:v0 - lo + nv] = gidx[b, v0:t0 + TOK]
        hsw = np.zeros((WIN, HID), dtype=np.float32)
        hsw[v0 - lo:v0 - lo + nv] = hs[b, v0:t0 + TOK]
        # hst8[kp*128+p, pl*WIN+c] = hs[g(c), 256kp+128pl+p]
        hst8 = np.ascontiguousarray(
            hsw.reshape(WIN, NKP, 2, 128).transpose(1, 3, 2, 0)
            .astype(_FP8).reshape(NKP * 128, 2 * WIN))
        in_maps.append({
            "tab8": tab8,
            "hst8": hst8,
            "wq8": wq8,
            "wvk8": wvk8,
            "idxs": np.ascontiguousarray(
                win_idx.reshape(NTILE, 128, NSLOT).transpose(1, 0, 2)
                .reshape(128, NTILE * NSLOT)),
        })
    return in_maps


def _postprocess_core(outT_np: np.ndarray, inputs: dict, c: int) -> np.ndarray:
    """Device outT (HID, TOK) bf16 fused -> full (TOK, HID) f32 output slice."""
    hs = np.asarray(inputs["hidden_states"], dtype=np.float32)
    cb = np.asarray(inputs["conv_b"], dtype=np.float32)
    b, h = divmod(c, 2)
    t0 = h * TOK
    return hs[b, t0:t0 + TOK, :] + outT_np.astype(np.float32).T + cb


def _run(inputs: dict, trace: bool = False, **kw):
    from concourse import bass_utils

    nc = _build_nc()
    in_maps = _make_in_maps(inputs)
    res = bass_utils.run_bass_kernel_spmd(
        nc, in_maps, core_ids=list(range(NCORES)), trace=trace, **kw)
    out = np.empty((B, S, HID), dtype=np.float32)
    for c in range(NCORES):
        b, h = divmod(c, 2)
        out[b, h * TOK:(h + 1) * TOK, :] = _postprocess_core(
            res.results[c]["outT"], inputs, c)
    return out, res


def kernel(**inputs) -> np.ndarray:
    out, _ = _run(inputs, trace=False)
    return out


# revision 5
# speedup vs baseline: 1.0902x; 1.0537x over previous
"""Trainium2 Bass kernel for the Engram module (hashed n-gram memory).

Contract: kernel(**inputs) takes FULL unsharded numpy inputs and returns the
FULL output (4, 2048, 2048) f32.

Sharding (hardcoded): data parallel over tokens — 8 cores x 1024 tokens
(core c -> batch c//2, seq half c%2); the 12 embedding tables replicated in
fp8 (x16 scale) in each core's DRAM; no collectives. Host computes the
n-gram hash indices (integer ops) while sharding; host also adds the f32
residual + conv bias during unshard.

Device pipeline per core (window of 1152 cols = 2 left-context + 1024 + pad):
  phase A (9 token tiles of 128):
    - 12 single-index indirect DMAs per tile gather the fp8 embedding rows
      (HW supports only one index per partition per indirect DMA)
    - q = hs @ W_q on PE, fp8 DoubleRow mode (2 K-planes per instruction)
    - dot(q, mem) via DVE mul+reduce; ACT sigmoid -> alpha; amb = alpha*mem
    - PE bf16 transposes into one PSUM bank; single ACT copy -> amt (fp8)
  phase B (2 passes x 16 hid tiles), causal conv FOLDED into the value
  matmul: 3 host-precomputed (W_v * conv_w[:,k]) fp8 matrices, 3 shifted
  rhs reads of amt, accumulated in PSUM via fp8 DoubleRow matmuls; ACT
  scale -> bf16 out; batched DMA out. The window is split L/R so the left
  pass's PE work overlaps the right half's gathers (Pool-engine bound).
"""

import os
import numpy as np
import ml_dtypes

# ---------------- problem constants (hardcoded per the contract) -------------
B, S, HID = 4, 2048, 2048
TABLE, EMB = 200000, 64
ORDERS, HEADS = 3, 4
NSLOT = ORDERS * HEADS            # 12
MEMD = NSLOT * EMB                # 768
KCONV = 3
NCORES = 8
TOK = 1024                        # output tokens per core
CTX = 2                           # left context (conv taps)
NTILE = 9                         # 9 uniform 128-token tiles
WIN = 128 * NTILE                 # 1152 cols; col c <-> token t0-2+c (pad tail)
VWIN = CTX + TOK                  # 1026 valid cols
ZROW = NSLOT * TABLE              # all-zeros pad row
TABROWS = ZROW + 4
NKP = HID // 256                  # 8 K-pair planes for Q
MKP = MEMD // 256                 # 3 K-pair planes for V
NHID = HID // 128                 # 16 hid tiles

SCALE_TAB = 16.0
SCALE_WQ = 32.0
SCALE_WVK = 128.0
SIG_SCALE = 1.0 / (float(np.sqrt(np.float64(MEMD))) * SCALE_TAB * SCALE_WQ)
OUT_SCALE = 1.0 / (SCALE_TAB * SCALE_WVK)

HEAD_MULTS = np.array([2654435761, 2246822519, 3266489917, 668265263],
                      dtype=np.uint32)
POLY = np.uint32(1000003)

_BF16 = ml_dtypes.bfloat16
_FP8 = ml_dtypes.float8_e4m3


def _global_rows(input_ids: np.ndarray) -> np.ndarray:
    """(B, S) int -> (B, S, 12) int32 global row ids into the stacked table."""
    Bb, Ss = input_ids.shape
    u = input_ids.astype(np.uint32)
    per_order = []
    for n in range(2, 2 + ORDERS):
        pad = np.zeros((Bb, Ss + n - 1), np.uint32)
        pad[:, n - 1:] = u
        acc = np.zeros((Bb, Ss), np.uint32)
        for j in range(n):
            acc = acc * POLY + pad[:, j:j + Ss]
        idx = (acc[..., None] * HEAD_MULTS[None, None, :]) % np.uint32(TABLE)
        per_order.append(idx.astype(np.int32))
    gidx = np.stack(per_order, axis=2).reshape(Bb, Ss, NSLOT)
    gidx = gidx + (np.arange(NSLOT, dtype=np.int32) * TABLE)[None, None, :]
    return gidx


# ---------------- device program ---------------------------------------------
_NC_CACHE: dict = {}


def _build_nc():
    _key = "nc" + os.environ.get("KPHASE", "AB")
    if _key in _NC_CACHE:
        return _NC_CACHE[_key]

    from contextlib import ExitStack

    import concourse.bass as bass
    import concourse.mybir as mybir
    import concourse.tile as tile
    from concourse import bacc, library_config
    from concourse.masks import make_identity

    f32 = mybir.dt.float32
    bf16 = mybir.dt.bfloat16
    fp8 = mybir.dt.float8e4
    i32 = mybir.dt.int32
    MULT = mybir.AluOpType.mult
    ADD = mybir.AluOpType.add
    AF = mybir.ActivationFunctionType
    AXF = mybir.AxisListType
    DR = mybir.MatmulPerfMode.DoubleRow

    nc = bacc.Bacc("TRN2", target_bir_lowering=False, debug=False,
                   enable_asserts=False, num_devices=NCORES)

    tab = nc.dram_tensor("tab8", [TABROWS, EMB], fp8,
                         kind="ExternalInput").ap()
    hst = nc.dram_tensor("hst8", [NKP * 128, 2 * WIN], fp8,
                         kind="ExternalInput").ap()
    wq = nc.dram_tensor("wq8", [NKP * 128, 2 * MEMD], fp8,
                        kind="ExternalInput").ap()
    wvk = nc.dram_tensor("wvk8", [MKP * 128, KCONV * 2 * HID], fp8,
                         kind="ExternalInput").ap()
    idxs = nc.dram_tensor("idxs", [128, NTILE * NSLOT], i32,
                          kind="ExternalInput").ap()
    outT = nc.dram_tensor("outT", [HID, TOK], bf16, kind="ExternalOutput").ap()

    with tile.TileContext(nc) as tc, ExitStack() as ctx:
        pool = lambda name, bufs, space="SBUF": ctx.enter_context(
            tc.tile_pool(name=name, bufs=bufs, space=space))

        p_const = pool("const", 1)
        p_w = pool("w", 1)
        p_amt = pool("amt", 1)
        p_mem = pool("mem", 6)
        p_amb = pool("amb", 2)
        p_scr = pool("scr", 2)
        p_sc = pool("sc", 3)
        p_out = pool("out", 2)
        p_qps = pool("qps", 2, space="PSUM")
        p_vps = pool("vps", 2, space="PSUM")

        identb = p_const.tile([128, 128], bf16)
        make_identity(nc, identb[:])
        # resident weights (one dma_start each; wvk split by K-pair) ---------
        idx_sb = p_w.tile([128, NTILE * NSLOT], i32, name="idx_sb")
        nc.sync.dma_start(idx_sb[:], idxs[:, :])
        hst_sb = p_w.tile([128, NKP, 2, WIN], fp8, name="hst_sb")
        nc.sync.dma_start(
            hst_sb[:],
            hst.rearrange("(kp p) (pl c) -> p kp pl c", p=128, pl=2))
        wq_sb = p_w.tile([128, NKP, 2, MEMD], fp8, name="wq_sb")
        nc.sync.dma_start(
            wq_sb[:],
            wq.rearrange("(kp p) (pl n) -> p kp pl n", p=128, pl=2))
        # wvk is loaded AFTER phase A is emitted (same SP queue) so the
        # hst/wq loads that gate the first Q matmul get the DMA bandwidth.
        wvk_sb = []
        for kp in range(MKP):
            t = p_w.tile([128, KCONV, 2, HID], fp8, name=f"wvk_sb{kp}")
            nc.sync.dma_start(
                t[:],
                wvk[128 * kp:128 * (kp + 1), :].rearrange(
                    "p (k pl h) -> p k pl h", k=KCONV, pl=2))
            wvk_sb.append(t)

        # window split for A/B overlap: pass p covers window cols
        # [256p, 256p+384); A tile t feeds every pass whose range it lies in.
        amtP = [p_amt.tile([128, MKP, 2, 384], fp8, name=f"amtP{p}")
                for p in range(4)]

        _phases = os.environ.get("KPHASE", "AB")
        if "A" not in _phases:
            for p in range(4):
                nc.vector.memset(amtP[p][:], 0.125)

        def _emit_bpass(p):
            src_t = amtP[p]
            for mt in range(NHID):
                h0 = 128 * mt
                ob = p_out.tile([128, 4, 256], bf16, tag=f"out{p}",
                                name=f"ob{p}_{mt // 4}_{mt % 4}") \
                    if mt % 4 == 0 else ob  # noqa: F821
                pv = p_vps.tile([128, 256], f32, space="PSUM", tag="v",
                                name=f"pv{p}_{mt}")
                n = 0
                for k in range(KCONV):
                    for kp in range(MKP):
                        nc.tensor.matmul(
                            pv[:, :],
                            lhsT=wvk_sb[kp][:, k, :, h0:h0 + 128],
                            rhs=src_t[:, kp, :, k:k + 256],
                            start=(n == 0), stop=(n == KCONV * MKP - 1),
                            perf_mode=DR)
                        n += 1
                nc.scalar.activation(ob[:, mt % 4, :], pv[:, :],
                                     AF.Identity, scale=OUT_SCALE)
                if mt % 4 == 3:
                    nc.sync.dma_start(
                        outT[512 * (mt // 4):512 * (mt // 4 + 1),
                             256 * p:256 * (p + 1)].rearrange(
                            "(sub p2) c -> p2 sub c", p2=128),
                        ob[:, :, :])

        # phase A: gather + gate + transposed alpha*mem ----------------------
        _pending = None
        _na = NTILE if ("A" in _phases or _phases in ("G", "Q")) else 0
        for i in range(_na):
            c0 = 128 * i
            w = min(128, WIN - c0)
            mem8 = p_mem.tile([128, MEMD], fp8, tag="mem", name=f"mem{i}")
            if _phases == "Q":
                nc.vector.memset(mem8[:], 0.125)
            else:
                for j in range(NSLOT):
                    nc.gpsimd.indirect_dma_start(
                        out=mem8[:, EMB * j:EMB * (j + 1)],
                        out_offset=None,
                        in_=tab[:, :],
                        in_offset=bass.IndirectOffsetOnAxis(
                            ap=idx_sb[:, NSLOT * i + j:NSLOT * i + j + 1],
                            axis=0))
            memf = p_scr.tile([128, MEMD], bf16, tag="memb", name=f"memb{i}")
            nc.scalar.activation(memf[:, :], mem8[:, :], AF.Identity)
            if _phases == "G":
                nc.scalar.activation(amt_sb[:, 0, 0, c0:c0 + w],
                                     memf[:, 0:w], AF.Identity)
                continue
            qA = p_qps.tile([128, 512], f32, space="PSUM", tag="qA", name=f"qA{i}")
            qB = p_qps.tile([128, 256], f32, space="PSUM", tag="qB", name=f"qB{i}")
            for n0 in range(0, MEMD, 256):
                qo = qA[0:w, n0:n0 + 256] if n0 < 512 else qB[0:w, 0:256]
                for kp in range(NKP):
                    nc.tensor.matmul(qo, lhsT=hst_sb[:, kp, :, c0:c0 + w],
                                     rhs=wq_sb[:, kp, :, n0:n0 + 256],
                                     start=(kp == 0), stop=(kp == NKP - 1),
                                     perf_mode=DR)
            prod = p_scr.tile([128, MEMD], f32, tag="scr", name=f"prod{i}")
            nc.vector.tensor_mul(prod[0:w, 0:512], qA[0:w, :], memf[0:w, 0:512])
            nc.vector.tensor_mul(prod[0:w, 512:768], qB[0:w, :],
                                 memf[0:w, 512:768])
            dot = p_sc.tile([128, 1], f32, tag="dot", name=f"dot{i}")
            nc.vector.tensor_reduce(dot[0:w, :], prod[0:w, :], AXF.X, ADD)
            alpha = p_sc.tile([128, 1], f32, tag="alpha", name=f"alpha{i}")
            nc.scalar.activation(alpha[0:w, :], dot[0:w, :], AF.Sigmoid,
                                 scale=SIG_SCALE)
            amb = p_amb.tile([128, MEMD], bf16, tag="amb", name=f"amb{i}")
            nc.scalar.activation(amb[0:w, :], memf[0:w, :], AF.Identity,
                                 scale=alpha[0:w, :])

            def _emit_transposes(amb, w, c0, i):
                # all 6 m-tiles transposed into ONE psum bank, one ACT copy
                tp = p_vps.tile([128, MEMD // 128, w], bf16, space="PSUM",
                                tag="v", name=f"tp{i}")
                for mt in range(MEMD // 128):
                    nc.tensor.transpose(
                        tp[:, mt, :], amb[0:w, 128 * mt:128 * (mt + 1)],
                        identb[0:w, 0:w])
                # ACT copy (DVE fp8 output is broken on HW); dest is the
                # [kp, pl] plane layout, source is m-tile-major == same order
                targets = [(amtP[p], c0 - 256 * p) for p in range(4)
                           if 256 * p <= c0 and c0 + 128 <= 256 * p + 384
                           and c0 < 256 * p + 258]
                for dst, cc in targets:
                    nc.scalar.activation(
                        dst[:, :, :, cc:cc + w].rearrange(
                            "p kp pl c -> p (kp pl) c"), tp[:, :, :],
                        AF.Identity)

            # software pipeline: transposes of tile i-1 go AFTER Q of tile i
            # in the PE queue, hiding the DVE/ACT gate latency.
            if _pending is not None:
                _emit_transposes(*_pending)
            _pending = (amb, w, c0, i)
            if i in (3, 5, 7) and "B" in _phases:
                _emit_bpass((i - 3) // 2)  # pass p ready after tile 2p+2
        if _pending is not None:
            _emit_transposes(*_pending)
        if "B" in _phases:
            _emit_bpass(3)

    nc.compile()
    _NC_CACHE[_key] = nc
    return nc


# ---------------- host-side sharding -----------------------------------------
def _make_in_maps(inputs: dict):
    hs = np.asarray(inputs["hidden_states"], dtype=np.float32)
    ids = np.asarray(inputs["input_ids"])
    tabs = np.asarray(inputs["emb_tables"], dtype=np.float32)
    W_q = np.asarray(inputs["W_q"], dtype=np.float32)
    W_v = np.asarray(inputs["W_v"], dtype=np.float32)
    conv_w = np.asarray(inputs["conv_w"], dtype=np.float32).reshape(HID, KCONV)
    conv_b = np.asarray(inputs["conv_b"], dtype=np.float32)

    tab8 = np.zeros((TABROWS, EMB), dtype=_FP8)
    tab8[:ZROW] = (tabs.reshape(ZROW, EMB) * SCALE_TAB).astype(_FP8)
    gidx = _global_rows(ids)                              # (B, S, 12) int32

    # wq8[kp*128+p, pl*768+n] = 32*W_q[256kp+128pl+p, n]
    wq8 = np.ascontiguousarray(
        (W_q.reshape(NKP, 2, 128, MEMD).transpose(0, 2, 1, 3) * SCALE_WQ)
        .astype(_FP8).reshape(NKP * 128, 2 * MEMD))
    # wvk8[kp*128+p, (k*2+pl)*2048+h] = 128*W_v[256kp+128pl+p, h]*conv_w[h,k]
    wvk = (W_v[None, :, :] * conv_w.T[:, None, :] * SCALE_WVK)  # (3, 768, 2048)
    wvk8 = np.ascontiguousarray(
        wvk.reshape(KCONV, MKP, 2, 128, HID).transpose(1, 3, 0, 2, 4)
        .astype(_FP8).reshape(MKP * 128, KCONV * 2 * HID))

    in_maps = []
    for c in range(NCORES):
        b, h = divmod(c, 2)
        t0 = h * TOK
        lo = t0 - CTX
        v0 = max(0, lo)                                   # first valid token
        nv = t0 + TOK - v0                                # valid token count
        win_idx = np.full((WIN, NSLOT), ZROW, dtype=np.int32)
        win_idx[v0 - lo:v0 - lo + nv] = gidx[b, v0:t0 + TOK]
        hsw = np.zeros((WIN, HID), dtype=np.float32)
        hsw[v0 - lo:v0 - lo + nv] = hs[b, v0:t0 + TOK]
        # hst8[kp*128+p, pl*WIN+c] = hs[g(c), 256kp+128pl+p]
        hst8 = np.ascontiguousarray(
            hsw.reshape(WIN, NKP, 2, 128).transpose(1, 3, 2, 0)
            .astype(_FP8).reshape(NKP * 128, 2 * WIN))
        in_maps.append({
            "tab8": tab8,
            "hst8": hst8,
            "wq8": wq8,
            "wvk8": wvk8,
            "idxs": np.ascontiguousarray(
                win_idx.reshape(NTILE, 128, NSLOT).transpose(1, 0, 2)
                .reshape(128, NTILE * NSLOT)),
        })
    return in_maps


def _postprocess_core(outT_np: np.ndarray, inputs: dict, c: int) -> np.ndarray:
    """Device outT (HID, TOK) bf16 fused -> full (TOK, HID) f32 output slice."""
    hs = np.asarray(inputs["hidden_states"], dtype=np.float32)
    cb = np.asarray(inputs["conv_b"], dtype=np.float32)
    b, h = divmod(c, 2)
    t0 = h * TOK
    return hs[b, t0:t0 + TOK, :] + outT_np.astype(np.float32).T + cb


def _run(inputs: dict, trace: bool = False, **kw):
    from concourse import bass_utils

    nc = _build_nc()
    in_maps = _make_in_maps(inputs)
    res = bass_utils.run_bass_kernel_spmd(
        nc, in_maps, core_ids=list(range(NCORES)), trace=trace, **kw)
    out = np.empty((B, S, HID), dtype=np.float32)
    for c in range(NCORES):
        b, h = divmod(c, 2)
        out[b, h * TOK:(h + 1) * TOK, :] = _postprocess_core(
            res.results[c]["outT"], inputs, c)
    return out, res


def kernel(**inputs) -> np.ndarray:
    out, _ = _run(inputs, trace=False)
    return out
